# revision 1
# baseline (speedup 1.0000x reference)
"""MultiLinearUpsampling Trainium2 kernel.

Problem: out[b, t, :] = W[lidx[t]] @ pooled[b, segc[t], :]  (zero for invalid t)
where segc/lidx derive from sorted pooling_indices (ragged segments).

Strategy
--------
Host computes the segment structure.  Only sum_l N_l matvecs are unique
per batch (N_l = #segments with len > l; positions past offset L-1 in a
segment reuse the l=L-1 result).  The device runs one SPMD program on 8
cores: P phases, phase p = one stationary weight plane (per-core data)
applied to C_p activation columns (per-core data, host-gathered).  A
small packing optimizer cuts the 16 linears' column sets into <=8
pieces per phase to minimize sum(C_p) (the per-core compute).  Matmuls
run in fp16 (full PE rate, ~3e-4 rel err for this data, half the DMA of
fp32); accumulation is fp32 in PSUM and outputs are fp32.  The host
scatters computed vectors to their t positions (including the l=15 tail
replication) and zero-fills invalid t.
"""

from contextlib import ExitStack

import numpy as np

import concourse.bass as bass  # noqa: F401  (bass types via bacc)
import concourse.mybir as mybir
import concourse.tile as tile
from concourse import bacc
from concourse.bass_utils import run_bass_kernel_spmd

F32 = mybir.dt.float32
F16 = mybir.dt.float16

B = 8          # batch (each core sees all batches)
N = 512        # segments
D = 1024       # D_in == D_out
L = 16         # linears
NCORES = 8
KC = 8         # contraction chunks of 128
MC = 8         # output-dim chunks of 128


# ---------------------------------------------------------------------------
# packing: choose phase sizes + piece assignment
# ---------------------------------------------------------------------------

def _combo_dp(sizes, Cs):
    """Assign each item (size) a piece-count vector over phase capacities Cs
    (max 8 pieces per phase) minimizing nothing fancy -- returns None if
    infeasible, else list of per-item count tuples."""
    P = len(Cs)
    items = list(sizes)
    combos_per_item = []
    for sz in items:
        combos = []
        max_counts = [min(8, -(-sz // c) if c else 0) for c in Cs]
        # enumerate small count vectors (total pieces <= 4)
        def rec(i, vec, cap):
            if sum(vec) > 4:
                return
            if i == P:
                if cap >= sz and sum(vec) > 0:
                    combos.append(tuple(vec))
                return
            for n in range(0, min(max_counts[i], 4) + 1):
                rec(i + 1, vec + [n], cap + n * Cs[i])

        rec(0, [], 0)
        if sz > 0 and not combos:
            return None
        combos_per_item.append(combos if sz > 0 else [tuple([0] * P)])

    # DP over cumulative piece counts in phases 0..P-2, minimize last phase
    from functools import lru_cache

    states = {tuple([0] * (P - 1)): 0}
    choice = []
    for combos in combos_per_item:
        nstates = {}
        back = {}
        for st, lastc in states.items():
            for cb in combos:
                nst = tuple(st[i] + cb[i] for i in range(P - 1))
                if any(v > 8 for v in nst):
                    continue
                nl = lastc + cb[P - 1]
                if nl > 8:
                    continue
                if nst not in nstates or nl < nstates[nst]:
                    nstates[nst] = nl
                    back[nst] = (st, cb)
        if not nstates:
            return None
        choice.append(back)
        states = nstates

    st = min(states, key=lambda s: states[s])
    picks = [None] * len(items)
    for i in range(len(items) - 1, -1, -1):
        st_prev, cb = choice[i][st]
        picks[i] = cb
        st = st_prev
    return picks


def _plan(N_l):
    """Return (Cs, slot_map): phase sizes and slot_map[c][p] =
    (l, col_start, cnt) or None."""
    order_l = np.argsort(-np.asarray(N_l), kind="stable")
    Ns = [int(N_l[i]) for i in order_l]
    total = sum(Ns)
    if total == 0:
        return [2], [[None] for _ in range(NCORES)]

    best = None  # (sumC, Cs, picks)

    def consider(Cs):
        nonlocal best
        Cs = [int(c) for c in Cs if c > 0]
        if not Cs:
            return
        if best is not None and sum(Cs) >= best[0]:
            return
        picks = _combo_dp(Ns, Cs)
        if picks is not None:
            best = (sum(Cs), Cs, picks)

    # baseline: heads unsplit at N(0), tails at N(8)
    c2 = Ns[8] if len(Ns) > 8 else 0
    consider([Ns[0], c2] if c2 else [Ns[0]])

    # precomputed optima for the benchmark's N_l (verified for the actual
    # data by the feasibility DP; harmless no-ops when infeasible)
    consider([214, 170, 110])
    consider([158, 132, 94, 80])

    if c2:
        head = Ns[:8]
        # family: tail phase at N(8); two head phases (X, Y) searched
        hi = head[0]
        for X in range(max(hi // 2, 64), hi + 1, 4):
            # minimal Y so every head item fits in <=3 pieces approx
            for Y in range(16, X + 1, 4):
                if 8 * (X + Y) < sum(head):
                    continue
                if best is not None and X + Y + c2 >= best[0]:
                    continue
                picks = _combo_dp(head, [X, Y])
                if picks is None:
                    continue
                # combine with tail phase
                consider([X, Y, c2])
                break  # smallest feasible Y for this X

    Cs = best[1]
    picks = best[2]
    # build slot map: phase -> list of pieces
    P = len(Cs)
    phase_pieces = [[] for _ in range(P)]
    for idx, l in enumerate(order_l):
        sz = Ns[idx]
        if sz == 0:
            continue
        pos = 0
        cb = picks[idx] if len(picks[idx]) == P else tuple(
            list(picks[idx]) + [0] * (P - len(picks[idx]))
        )
        for p in range(P):
            for _ in range(cb[p]):
                cnt = min(Cs[p], sz - pos)
                if cnt <= 0:
                    continue
                phase_pieces[p].append((int(l), pos, cnt))
                pos += cnt
        assert pos >= sz, f"l={l} not covered: {pos}/{sz}"

    slot_map = [[None] * P for _ in range(NCORES)]
    for p in range(P):
        assert len(phase_pieces[p]) <= NCORES, (p, phase_pieces[p])
        for c, piece in enumerate(phase_pieces[p]):
            slot_map[c][p] = piece
    return Cs, slot_map


# ---------------------------------------------------------------------------
# device program
# ---------------------------------------------------------------------------

def _build_program(Cs):
    """Inputs: x{p} (D, B, C_p) f16, wt (P, D, D) f16 (wt[p] = W-plane.T).
    Outputs: y{p} (B, D, C_p) f32."""
    nc = bacc.Bacc("TRN2", target_bir_lowering=False, debug=False)
    P = len(Cs)
    CT = sum(Cs)

    xs = [
        nc.dram_tensor(f"x{p}", (D, B, C), F16, kind="ExternalInput")
        for p, C in enumerate(Cs)
    ]
    wt = nc.dram_tensor("wt", (P, D, D), F16, kind="ExternalInput")
    ys = [
        nc.dram_tensor(f"y{p}", (B, D, C), F32, kind="ExternalOutput")
        for p, C in enumerate(Cs)
    ]

    # (kp, kc, b*c) views: per-k rows are contiguous B*C_p runs -> 2D DMAs
    xs_r = [
        x.ap().rearrange("(kc kp) b n -> kp kc (b n)", kp=128) for x in xs
    ]
    wt_r = wt.ap().rearrange("p (kc kp) m -> kp p kc m", kp=128)

    # process phases smallest-first: the first phase's inputs arrive
    # quickly, later phases' inputs stream in behind its compute
    order_p = sorted(range(P), key=lambda p: Cs[p])

    with tile.TileContext(nc) as tc, ExitStack() as ctx:
        wpool = ctx.enter_context(tc.tile_pool(name="w", bufs=1))
        xpool = ctx.enter_context(tc.tile_pool(name="x", bufs=1))
        opool = ctx.enter_context(tc.tile_pool(name="o", bufs=3))
        ppool = ctx.enter_context(tc.tile_pool(name="ps", bufs=6, space="PSUM"))

        # resident inputs, emitted in consumption order (few large DMAs:
        # each dma_start costs ~600ns of sequencer issue time regardless
        # of size, so instruction count matters more than granularity).
        # W goes on the otherwise-idle Scalar queue so X and W issue in
        # parallel instead of serializing on the Sync sequencer.
        wtiles = {}
        xtiles = {}

        def emit_w(p):
            for k in range(KC):
                wtiles[p, k] = wpool.tile(
                    [128, D], F16, tag=f"w{p}_{k}", name=f"w{p}_{k}"
                )
                nc.sync.dma_start(wtiles[p, k][:], wt_r[:, p, k])

        def emit_x(p):
            C = Cs[p]
            for k in range(KC):
                xtiles[p, k] = xpool.tile(
                    [128, B, C], F16, tag=f"x{p}_{k}", name=f"x{p}_{k}"
                )
                nc.sync.dma_start(
                    xtiles[p, k][:].rearrange("kp b n -> kp (b n)"), xs_r[p][:, k]
                )

        for p in order_p:
            emit_x(p)
            emit_w(p)

        for p in order_p:
            C = Cs[p]
            g = max(1, min(B, 512 // C))
            for b0 in range(0, B, g):
                gg = min(g, B - b0)
                ot = opool.tile([128, MC, gg, C], F32, tag="o")
                for m in range(MC):
                    ps = ppool.tile([128, gg, C], F32, tag="ps")
                    for k in range(KC):
                        nc.tensor.matmul(
                            ps[:],
                            wtiles[p, k][:, m * 128 : (m + 1) * 128],
                            xtiles[p, k][:, b0 : b0 + gg, :],
                            start=(k == 0),
                            stop=(k == KC - 1),
                        )
                    nc.vector.tensor_copy(ot[:, m], ps[:])
                # one output DMA per batch row (GpSimd queue: keeps the
                # Sync sequencer free for input issue)
                for bi in range(gg):
                    nc.gpsimd.dma_start(
                        ys[p][b0 + bi].rearrange("(m kp) c -> kp m c", kp=128),
                        ot[:, :, bi, :],
                    )

    nc.compile()
    return nc


# ---------------------------------------------------------------------------
# host wrapper
# ---------------------------------------------------------------------------

def _segment_structure(idx, T):
    t = np.arange(T)
    seg = np.searchsorted(idx, t, side="left")
    valid = seg < N
    segc = np.clip(seg, 0, N - 1)
    start = np.where(segc > 0, idx[np.maximum(segc - 1, 0)] + 1, 0)
    lidx = np.minimum(t - start, L - 1).astype(np.int64)
    lens = np.bincount(segc[valid], minlength=N)
    return t, seg, valid, segc, lidx, lens


def _install_ntff_hook():
    """Profiling-only: register the axon NTFF profile hook (dev use)."""
    import sys
    import types

    try:
        import antenv

        if "antenv.axon_hooks" not in sys.modules:
            mod = types.ModuleType("antenv.axon_hooks")
            holder = [None]
            mod.set_axon_ntff_profile_hook = lambda h: holder.__setitem__(0, h)
            mod.get_axon_ntff_profile_hook = lambda: holder[0]
            sys.modules["antenv.axon_hooks"] = mod
            antenv.axon_hooks = mod
            from trn_agent_boot.trn_boot import _ntff_profile_via_ctypes

            mod.set_axon_ntff_profile_hook(
                _ntff_profile_via_ctypes("/opt/axon/libaxon_pjrt.so")
            )
    except Exception as e:
        print(f"NTFF hook install failed: {e}")


def kernel(pooled_vectors, W, pooling_indices, target_length, _trace=False):
    pooled = np.asarray(pooled_vectors, dtype=np.float32)
    Wf = np.asarray(W, dtype=np.float32)
    idx = np.asarray(pooling_indices).astype(np.int64)
    T = int(np.asarray(target_length))

    t, seg, valid, segc, lidx, lens = _segment_structure(idx, T)

    order = np.argsort(-lens, kind="stable")
    rank_of_seg = np.empty(N, dtype=np.int64)
    rank_of_seg[order] = np.arange(N)
    N_l = (lens[None, :] > np.arange(L)[:, None]).sum(axis=1)

    Cs, slot_map = _plan(N_l)
    P = len(Cs)

    nc = _build_program(Cs)

    # host-side gathered inputs, fp16
    Xg = np.ascontiguousarray(pooled.transpose(2, 0, 1)[:, :, order]).astype(
        np.float16
    )  # (D, B, N) sorted columns
    Wt16 = np.ascontiguousarray(Wf.transpose(0, 2, 1)).astype(np.float16)  # (L,D,D) .T

    in_maps = []
    for c in range(NCORES):
        wt_c = np.zeros((P, D, D), dtype=np.float16)
        im = {}
        for p in range(P):
            xp = np.zeros((D, B, Cs[p]), dtype=np.float16)
            piece = slot_map[c][p]
            if piece is not None:
                l, c0, cnt = piece
                xp[:, :, :cnt] = Xg[:, :, c0 : c0 + cnt]
                wt_c[p] = Wt16[l]
            im[f"x{p}"] = xp
        im["wt"] = wt_c
        in_maps.append(im)

    kwargs = {}
    if _trace:
        _install_ntff_hook()
        kwargs = dict(trace=True)
    res = run_bass_kernel_spmd(nc, in_maps, core_ids=list(range(NCORES)), **kwargs)
    results = res.results

    # per-(l, col-rank) -> (core, phase, j) maps
    maxN = int(N_l.max()) if L else 0
    core_of = np.full((L, max(maxN, 1)), -1, dtype=np.int32)
    phase_of = np.zeros((L, max(maxN, 1)), dtype=np.int32)
    j_of = np.zeros((L, max(maxN, 1)), dtype=np.int32)
    for c in range(NCORES):
        for p in range(P):
            piece = slot_map[c][p]
            if piece is None:
                continue
            l, c0, cnt = piece
            core_of[l, c0 : c0 + cnt] = c
            phase_of[l, c0 : c0 + cnt] = p
            j_of[l, c0 : c0 + cnt] = np.arange(cnt)

    Dout = Wf.shape[1]
    out = np.zeros((B, T, Dout), dtype=np.float32)
    tv = t[valid]
    l_t = lidx[valid]
    r_t = rank_of_seg[segc[valid]]
    ct = core_of[l_t, r_t]
    pt = phase_of[l_t, r_t]
    jt = j_of[l_t, r_t]
    assert (ct >= 0).all(), "uncovered (l, col) in assignment"

    for p in range(P):
        sel = pt == p
        if not sel.any():
            continue
        Yp = np.stack([results[c][f"y{p}"] for c in range(NCORES)])  # (8,B,D,C_p)
        out[:, tv[sel], :] = Yp[ct[sel], :, :, jt[sel]].transpose(1, 0, 2)

    if _trace:
        kernel._last_exec_time_ns = res.exec_time_ns
        kernel._last_results = res
    return out



# revision 4
# speedup vs baseline: 1.1043x; 1.1043x over previous
"""MultiLinearUpsampling Trainium2 kernel.

Problem: out[b, t, :] = W[lidx[t]] @ pooled[b, segc[t], :]  (zero for invalid t)
where segc/lidx derive from sorted pooling_indices (ragged segments).

Strategy (v2: output-dim split)
-------------------------------
Only sum_l N_l matvecs are unique per batch (N_l = #segments with
len > l).  Sorting segments by length (desc) makes each linear l's
column set a PREFIX of one flat (rank, batch) column axis: linear l
applies to flat columns [0, 8*N_l).

Each of the 8 cores owns a 128-row slice of D_out and computes ALL
columns for ALL 16 linears on that slice: perfectly balanced, no
packing waste, and X is a single shared gather.  Per core per l:
Y_l[m, c] = sum_d W[l, m_slice, d] * X[d, c] for c < 8*N_l, computed
as 8 contraction tiles x <=512-wide PSUM windows, fp16 in / fp32
accumulate / fp16 out.  l runs smallest-prefix first so early X
chunks enable compute immediately and outputs drain throughout.

The host scatters Y columns to their t positions (including the
l = L-1 tail replication) and zero-fills invalid t.
"""

from contextlib import ExitStack

import numpy as np

import concourse.bass as bass  # noqa: F401  (bass types via bacc)
import concourse.mybir as mybir
import concourse.tile as tile
from concourse import bacc
from concourse.bass_utils import run_bass_kernel_spmd

F32 = mybir.dt.float32
F16 = mybir.dt.float16

B = 8          # batch
N = 512        # segments
D = 1024       # D_in == D_out
L = 16         # linears
NCORES = 8
KC = 8         # contraction chunks of 128
MSLICE = 128   # out-dim rows per core
WMAX = 512     # PSUM window width (one bank of fp32)


def _windows(F_l, bounds):
    """Split [0, F_l) at region bounds and into balanced <=WMAX chunks.
    Returns list of (region_index, start, len) with start relative to
    the region start."""
    out = []
    for ri in range(len(bounds) - 1):
        lo, hi = bounds[ri], min(bounds[ri + 1], F_l)
        if hi <= lo:
            break
        width = hi - lo
        nw = -(-width // WMAX)
        base, rem = divmod(width, nw)
        off = 0
        for j in range(nw):
            ln = base + (1 if j < rem else 0)
            out.append((ri, lo - bounds[ri] + off, ln))
            off += ln
    return out


def _build_program(F_ls, bounds):
    """F_ls: per-linear flat-column prefix lengths (ascending process
    order, zeros removed).  bounds: region boundaries [0, b1, .., F]."""
    nc = bacc.Bacc("TRN2", target_bir_lowering=False, debug=False)
    nreg = len(bounds) - 1
    widths = [bounds[i + 1] - bounds[i] for i in range(nreg)]
    F = bounds[-1]
    NL = len(F_ls)
    total_cols = sum(F_ls)

    xs = [
        nc.dram_tensor(f"x{r}", (KC, 128, widths[r]), F16, kind="ExternalInput")
        for r in range(nreg)
    ]
    wh = nc.dram_tensor("w", (NL, 128, KC * 128), F16, kind="ExternalInput")
    y = nc.dram_tensor("y", (128, total_cols), F16, kind="ExternalOutput")

    with tile.TileContext(nc) as tc, ExitStack() as ctx:
        xpool = ctx.enter_context(tc.tile_pool(name="x", bufs=1))
        wpool = ctx.enter_context(tc.tile_pool(name="w", bufs=1))
        ypool = ctx.enter_context(tc.tile_pool(name="y", bufs=3))
        ppool = ctx.enter_context(tc.tile_pool(name="ps", bufs=8, space="PSUM"))

        # X: per (region, k) tiles so compute can start as soon as the
        # first contraction tile of the first region lands (HWDGE sync
        # ring).  W per l on the scalar HWDGE ring, issued in process
        # order, so both input streams issue in parallel.
        xt = {}
        for r in range(nreg):
            for k in range(KC):
                xt[r, k] = xpool.tile(
                    [128, widths[r]], F16, tag=f"x{r}_{k}", name=f"x{r}_{k}"
                )
                nc.sync.dma_start(xt[r, k][:], xs[r].ap()[k])
        wt = {}
        for li in range(NL):
            wt[li] = wpool.tile([128, KC * 128], F16, tag=f"w{li}", name=f"w{li}")
            nc.scalar.dma_start(wt[li][:], wh.ap()[li])

        off = 0
        for li in range(NL):
            F_l = F_ls[li]
            wins = _windows(F_l, bounds)
            yt = ypool.tile([128, F_l], F16, tag="yt", name=f"y{li}")
            # groups of <=4 windows: k-outer within a group (stationary
            # reuse, PSUM half-rotation so copies overlap next group)
            for g0 in range(0, len(wins), 4):
                grp = wins[g0 : g0 + 4]
                pts = [
                    ppool.tile([128, WMAX], F32, tag="ps", name=f"ps{li}_{g0}_{j}")
                    for j in range(len(grp))
                ]
                for k in range(KC):
                    for (ri, ws, wl), pt in zip(grp, pts):
                        nc.tensor.matmul(
                            pt[:, :wl],
                            wt[li][:, k * 128 : (k + 1) * 128],
                            xt[ri, k][:, ws : ws + wl],
                            start=(k == 0),
                            stop=(k == KC - 1),
                        )
                for (ri, ws, wl), pt in zip(grp, pts):
                    woff = bounds[ri] + ws
                    nc.vector.tensor_copy(yt[:, woff : woff + wl], pt[:, :wl])
            nc.gpsimd.dma_start(y.ap()[:, off : off + F_l], yt[:])
            off += F_l

    nc.compile()
    return nc


# ---------------------------------------------------------------------------
# host wrapper
# ---------------------------------------------------------------------------

def _segment_structure(idx, T):
    t = np.arange(T)
    seg = np.searchsorted(idx, t, side="left")
    valid = seg < N
    segc = np.clip(seg, 0, N - 1)
    start = np.where(segc > 0, idx[np.maximum(segc - 1, 0)] + 1, 0)
    lidx = np.minimum(t - start, L - 1).astype(np.int64)
    lens = np.bincount(segc[valid], minlength=N)
    return t, seg, valid, segc, lidx, lens


def _install_ntff_hook():
    """Profiling-only: register the axon NTFF profile hook (dev use)."""
    import sys
    import types

    try:
        import antenv

        if "antenv.axon_hooks" not in sys.modules:
            mod = types.ModuleType("antenv.axon_hooks")
            holder = [None]
            mod.set_axon_ntff_profile_hook = lambda h: holder.__setitem__(0, h)
            mod.get_axon_ntff_profile_hook = lambda: holder[0]
            sys.modules["antenv.axon_hooks"] = mod
            antenv.axon_hooks = mod
            from trn_agent_boot.trn_boot import _ntff_profile_via_ctypes

            mod.set_axon_ntff_profile_hook(
                _ntff_profile_via_ctypes("/opt/axon/libaxon_pjrt.so")
            )
    except Exception as e:
        print(f"NTFF hook install failed: {e}")


def kernel(pooled_vectors, W, pooling_indices, target_length, _trace=False):
    pooled = np.asarray(pooled_vectors, dtype=np.float32)
    Wf = np.asarray(W, dtype=np.float32)
    idx = np.asarray(pooling_indices).astype(np.int64)
    T = int(np.asarray(target_length))

    t, seg, valid, segc, lidx, lens = _segment_structure(idx, T)

    order = np.argsort(-lens, kind="stable")      # segments by len desc
    rank_of_seg = np.empty(N, dtype=np.int64)
    rank_of_seg[order] = np.arange(N)
    N_l = (lens[None, :] > np.arange(L)[:, None]).sum(axis=1)  # (L,)

    # process order: ascending prefix length, zero-size linears skipped
    proc = [l for l in np.argsort(N_l, kind="stable") if N_l[l] > 0]
    F_ls = [8 * int(N_l[l]) for l in proc]
    F = max(F_ls) if F_ls else 8
    # region boundaries: first ~4 linears entirely in region A, ~12 in
    # A+B, so early compute needs only early X chunks
    cand = {F}
    if len(F_ls) > 4:
        cand.add(F_ls[3])
    if len(F_ls) > 12:
        cand.add(F_ls[11])
    bounds = [0] + sorted(cand)

    nc = _build_program(F_ls, bounds)

    # flat column axis: (rank-major, batch-minor), ranks with len>0 only
    n0 = F // 8
    Xh = (
        pooled.transpose(2, 1, 0)[:, order[:n0], :]
        .reshape(D, F)
        .astype(np.float16)
    )  # (D, F), col = r*8 + b
    xregions = [
        np.ascontiguousarray(
            Xh[:, bounds[r] : bounds[r + 1]].reshape(KC, 128, -1)
        )
        for r in range(len(bounds) - 1)
    ]

    in_maps = []
    for c in range(NCORES):
        # W[l, m_slice, d] -> (l, kp, kc*128+m) with d = kc*128 + kp
        wc = (
            Wf[np.array(proc), c * 128 : (c + 1) * 128, :]
            .transpose(0, 2, 1)
            .reshape(len(proc), KC, 128, 128)
            .transpose(0, 2, 1, 3)
            .reshape(len(proc), 128, KC * 128)
            .astype(np.float16)
        )
        im = {"w": np.ascontiguousarray(wc)}
        for r, xr in enumerate(xregions):
            im[f"x{r}"] = xr
        in_maps.append(im)

    kwargs = {}
    if _trace:
        _install_ntff_hook()
        kwargs = dict(trace=True)
    res = run_bass_kernel_spmd(nc, in_maps, core_ids=list(range(NCORES)), **kwargs)
    results = res.results

    # assemble (D, total_cols) then scatter to (B, T, D)
    Yall = np.concatenate(
        [np.asarray(results[c]["y"]) for c in range(NCORES)], axis=0
    )  # (1024, total_cols) f16
    col_off = np.zeros(L, dtype=np.int64)
    off = 0
    for li, l in enumerate(proc):
        col_off[l] = off
        off += F_ls[li]

    Dout = Wf.shape[1]
    out = np.zeros((B, T, Dout), dtype=np.float32)
    tv = t[valid]
    ci = col_off[lidx[tv]] + rank_of_seg[segc[tv]] * 8  # (Tv,)
    cib = ci[:, None] + np.arange(B)[None, :]           # (Tv, B)
    out[:, tv, :] = Yall[:, cib].transpose(2, 1, 0).astype(np.float32)

    if _trace:
        kernel._last_exec_time_ns = res.exec_time_ns
        kernel._last_results = res
    return out


# revision 7
# speedup vs baseline: 1.1435x; 1.0355x over previous
"""MultiLinearUpsampling Trainium2 kernel.

Problem: out[b, t, :] = W[lidx[t]] @ pooled[b, segc[t], :]  (zero for invalid t)
where segc/lidx derive from sorted pooling_indices (ragged segments).

Strategy (v2: output-dim split)
-------------------------------
Only sum_l N_l matvecs are unique per batch (N_l = #segments with
len > l).  Sorting segments by length (desc) makes each linear l's
column set a PREFIX of one flat (rank, batch) column axis: linear l
applies to flat columns [0, 8*N_l).

Each of the 8 cores owns a 128-row slice of D_out and computes ALL
columns for ALL 16 linears on that slice: perfectly balanced, no
packing waste, and X is a single shared gather.  Per core per l:
Y_l[m, c] = sum_d W[l, m_slice, d] * X[d, c] for c < 8*N_l, computed
as 8 contraction tiles x <=512-wide PSUM windows, fp16 in / fp32
accumulate / fp16 out.  l runs smallest-prefix first so early X
chunks enable compute immediately and outputs drain throughout.

The host scatters Y columns to their t positions (including the
l = L-1 tail replication) and zero-fills invalid t.
"""

from contextlib import ExitStack

import numpy as np

import concourse.bass as bass  # noqa: F401  (bass types via bacc)
import concourse.mybir as mybir
import concourse.tile as tile
from concourse import bacc
from concourse.bass_utils import run_bass_kernel_spmd

F32 = mybir.dt.float32
F16 = mybir.dt.float16

B = 8          # batch
N = 512        # segments
D = 1024       # D_in == D_out
L = 16         # linears
NCORES = 8
KC = 8         # contraction chunks of 128
MSLICE = 128   # out-dim rows per core
WMAX = 512     # PSUM window width (one bank of fp32)


def _windows(F_l, bounds):
    """Split [0, F_l) at region bounds and into balanced <=WMAX chunks.
    Returns list of (region_index, start, len) with start relative to
    the region start."""
    out = []
    for ri in range(len(bounds) - 1):
        lo, hi = bounds[ri], min(bounds[ri + 1], F_l)
        if hi <= lo:
            break
        width = hi - lo
        nw = -(-width // WMAX)
        base, rem = divmod(width, nw)
        off = 0
        for j in range(nw):
            ln = base + (1 if j < rem else 0)
            out.append((ri, lo - bounds[ri] + off, ln))
            off += ln
    return out


def _build_program(F_ls, bounds):
    """F_ls: per-linear flat-column prefix lengths (ascending process
    order, zeros removed).  bounds: region boundaries [0, b1, .., F]."""
    nc = bacc.Bacc("TRN2", target_bir_lowering=False, debug=False)
    nreg = len(bounds) - 1
    widths = [bounds[i + 1] - bounds[i] for i in range(nreg)]
    F = bounds[-1]
    NL = len(F_ls)
    total_cols = sum(F_ls)

    xs = [
        nc.dram_tensor(f"x{r}", (KC, 128, widths[r]), F16, kind="ExternalInput")
        for r in range(nreg)
    ]
    wh = nc.dram_tensor("w", (NL, 128, KC * 128), F16, kind="ExternalInput")
    y = nc.dram_tensor("y", (128, total_cols), F16, kind="ExternalOutput")

    with tile.TileContext(nc) as tc, ExitStack() as ctx:
        xpool = ctx.enter_context(tc.tile_pool(name="x", bufs=1))
        wpool = ctx.enter_context(tc.tile_pool(name="w", bufs=1))
        ypool = ctx.enter_context(tc.tile_pool(name="y", bufs=3))
        ppool = ctx.enter_context(tc.tile_pool(name="ps", bufs=8, space="PSUM"))

        # PE clock warm-up: the p-state ramps only under sustained matmul
        # activity, so burn a few dummy matmuls on scratch tiles while the
        # first real inputs are still in flight.
        sx = xpool.tile([128, WMAX], F16, tag="scratch_x", name="sx")
        sw = xpool.tile([128, 128], F16, tag="scratch_w", name="sw")
        nc.vector.memset(sx[:], 0.0)
        nc.vector.memset(sw[:], 0.0)
        spt = ppool.tile([128, WMAX], F32, tag="ps", name="spt")
        for _ in range(10):
            nc.tensor.matmul(spt[:], sw[:], sx[:], start=True, stop=True)

        # X: one fat DMA per region on the sync HWDGE ring (per-DMA fixed
        # cost ~0.6us, transfers serialize per ring -> few big transfers).
        # Region 0's k=0 slice goes first so compute starts early.
        # W per l on the scalar HWDGE ring, issued in process order, so
        # both input streams issue in parallel.
        xt = {}
        for r in range(nreg):
            xt[r] = xpool.tile([128, KC, widths[r]], F16, tag=f"x{r}", name=f"x{r}")
        xa0 = xpool.tile([128, widths[0]], F16, tag="xa0", name="xa0")
        src0 = xs[0].ap().rearrange("kc kp w -> kp kc w")
        nc.sync.dma_start(xa0[:], src0[:, 0])
        nc.sync.dma_start(xt[0][:, 1:], src0[:, 1:])
        for r in range(1, nreg):
            nc.sync.dma_start(
                xt[r][:], xs[r].ap().rearrange("kc kp w -> kp kc w")
            )
        wt = {}
        for li in range(NL):
            wt[li] = wpool.tile([128, KC * 128], F16, tag=f"w{li}", name=f"w{li}")
            nc.scalar.dma_start(wt[li][:], wh.ap()[li])

        def moving(ri, k, ws, wl):
            if ri == 0 and k == 0:
                return xa0[:, ws : ws + wl]
            return xt[ri][:, k, ws : ws + wl]

        off = 0
        for li in range(NL):
            F_l = F_ls[li]
            wins = _windows(F_l, bounds)
            yt = ypool.tile([128, F_l], F16, tag="yt", name=f"y{li}")
            # groups of <=4 windows: k-outer within a group (stationary
            # reuse, PSUM half-rotation so copies overlap next group)
            ydrain = 0
            for g0 in range(0, len(wins), 4):
                grp = wins[g0 : g0 + 4]
                pts = [
                    ppool.tile([128, WMAX], F32, tag="ps", name=f"ps{li}_{g0}_{j}")
                    for j in range(len(grp))
                ]
                for k in range(KC):
                    for (ri, ws, wl), pt in zip(grp, pts):
                        nc.tensor.matmul(
                            pt[:, :wl],
                            wt[li][:, k * 128 : (k + 1) * 128],
                            moving(ri, k, ws, wl),
                            start=(k == 0),
                            stop=(k == KC - 1),
                        )
                for (ri, ws, wl), pt in zip(grp, pts):
                    woff = bounds[ri] + ws
                    nc.vector.tensor_copy(yt[:, woff : woff + wl], pt[:, :wl])
                # drain the first half of a big block early so the tail
                # DMA is small
                done = bounds[grp[-1][0]] + grp[-1][1] + grp[-1][2]
                if ydrain == 0 and F_l >= 3000 and F_l // 2 <= done < F_l:
                    nc.gpsimd.dma_start(
                        y.ap()[:, off : off + done], yt[:, :done]
                    )
                    ydrain = done
            nc.gpsimd.dma_start(
                y.ap()[:, off + ydrain : off + F_l], yt[:, ydrain:]
            )
            off += F_l

    nc.compile()
    return nc


# ---------------------------------------------------------------------------
# host wrapper
# ---------------------------------------------------------------------------

def _segment_structure(idx, T):
    t = np.arange(T)
    seg = np.searchsorted(idx, t, side="left")
    valid = seg < N
    segc = np.clip(seg, 0, N - 1)
    start = np.where(segc > 0, idx[np.maximum(segc - 1, 0)] + 1, 0)
    lidx = np.minimum(t - start, L - 1).astype(np.int64)
    lens = np.bincount(segc[valid], minlength=N)
    return t, seg, valid, segc, lidx, lens


def _install_ntff_hook():
    """Profiling-only: register the axon NTFF profile hook (dev use)."""
    import sys
    import types

    try:
        import antenv

        if "antenv.axon_hooks" not in sys.modules:
            mod = types.ModuleType("antenv.axon_hooks")
            holder = [None]
            mod.set_axon_ntff_profile_hook = lambda h: holder.__setitem__(0, h)
            mod.get_axon_ntff_profile_hook = lambda: holder[0]
            sys.modules["antenv.axon_hooks"] = mod
            antenv.axon_hooks = mod
            from trn_agent_boot.trn_boot import _ntff_profile_via_ctypes

            mod.set_axon_ntff_profile_hook(
                _ntff_profile_via_ctypes("/opt/axon/libaxon_pjrt.so")
            )
    except Exception as e:
        print(f"NTFF hook install failed: {e}")


def kernel(pooled_vectors, W, pooling_indices, target_length, _trace=False):
    pooled = np.asarray(pooled_vectors, dtype=np.float32)
    Wf = np.asarray(W, dtype=np.float32)
    idx = np.asarray(pooling_indices).astype(np.int64)
    T = int(np.asarray(target_length))

    t, seg, valid, segc, lidx, lens = _segment_structure(idx, T)

    order = np.argsort(-lens, kind="stable")      # segments by len desc
    rank_of_seg = np.empty(N, dtype=np.int64)
    rank_of_seg[order] = np.arange(N)
    N_l = (lens[None, :] > np.arange(L)[:, None]).sum(axis=1)  # (L,)

    # process order: ascending prefix length (early compute only needs
    # early X chunks), except the smallest linear moves to the end so
    # the final output drain is tiny; zero-size linears skipped
    proc = [l for l in np.argsort(N_l, kind="stable") if N_l[l] > 0]
    if len(proc) > 2:
        proc = proc[1:] + proc[:1]
    F_ls = [8 * int(N_l[l]) for l in proc]
    F = max(F_ls) if F_ls else 8
    # region boundaries: first ~4 processed linears entirely in region
    # A, ~12 in A+B
    srt = sorted(F_ls)
    cand = {F}
    if len(srt) > 4:
        cand.add(srt[3])
    if len(srt) > 12:
        cand.add(srt[11])
    bounds = [0] + sorted(cand)

    nc = _build_program(F_ls, bounds)

    # flat column axis: (rank-major, batch-minor), ranks with len>0 only
    n0 = F // 8
    Xh = (
        pooled.transpose(2, 1, 0)[:, order[:n0], :]
        .reshape(D, F)
        .astype(np.float16)
    )  # (D, F), col = r*8 + b
    xregions = [
        np.ascontiguousarray(
            Xh[:, bounds[r] : bounds[r + 1]].reshape(KC, 128, -1)
        )
        for r in range(len(bounds) - 1)
    ]

    in_maps = []
    for c in range(NCORES):
        # W[l, m_slice, d] -> (l, kp, kc*128+m) with d = kc*128 + kp
        wc = (
            Wf[np.array(proc), c * 128 : (c + 1) * 128, :]
            .transpose(0, 2, 1)
            .reshape(len(proc), KC, 128, 128)
            .transpose(0, 2, 1, 3)
            .reshape(len(proc), 128, KC * 128)
            .astype(np.float16)
        )
        im = {"w": np.ascontiguousarray(wc)}
        for r, xr in enumerate(xregions):
            im[f"x{r}"] = xr
        in_maps.append(im)

    kwargs = {}
    if _trace:
        _install_ntff_hook()
        kwargs = dict(trace=True)
    res = run_bass_kernel_spmd(nc, in_maps, core_ids=list(range(NCORES)), **kwargs)
    results = res.results

    # assemble (D, total_cols) then scatter to (B, T, D)
    Yall = np.concatenate(
        [np.asarray(results[c]["y"]) for c in range(NCORES)], axis=0
    )  # (1024, total_cols) f16
    col_off = np.zeros(L, dtype=np.int64)
    off = 0
    for li, l in enumerate(proc):
        col_off[l] = off
        off += F_ls[li]

    Dout = Wf.shape[1]
    out = np.zeros((B, T, Dout), dtype=np.float32)
    tv = t[valid]
    ci = col_off[lidx[tv]] + rank_of_seg[segc[tv]] * 8  # (Tv,)
    cib = ci[:, None] + np.arange(B)[None, :]           # (Tv, B)
    out[:, tv, :] = Yall[:, cib].transpose(2, 1, 0).astype(np.float32)

    if _trace:
        kernel._last_exec_time_ns = res.exec_time_ns
        kernel._last_results = res
    return out


# revision 9
# speedup vs baseline: 1.2238x; 1.0702x over previous
"""MultiLinearUpsampling Trainium2 kernel.

Problem: out[b, t, :] = W[lidx[t]] @ pooled[b, segc[t], :]  (zero for invalid t)
where segc/lidx derive from sorted pooling_indices (ragged segments).

Strategy (v2: output-dim split)
-------------------------------
Only sum_l N_l matvecs are unique per batch (N_l = #segments with
len > l).  Sorting segments by length (desc) makes each linear l's
column set a PREFIX of one flat (rank, batch) column axis: linear l
applies to flat columns [0, 8*N_l).

Each of the 8 cores owns a 128-row slice of D_out and computes ALL
columns for ALL 16 linears on that slice: perfectly balanced, no
packing waste, and X is a single shared gather.  Per core per l:
Y_l[m, c] = sum_d W[l, m_slice, d] * X[d, c] for c < 8*N_l, computed
as 8 contraction tiles x <=512-wide PSUM windows, fp16 in / fp32
accumulate / fp16 out.  l runs smallest-prefix first so early X
chunks enable compute immediately and outputs drain throughout.

The host scatters Y columns to their t positions (including the
l = L-1 tail replication) and zero-fills invalid t.
"""

from contextlib import ExitStack

import numpy as np

import concourse.bass as bass  # noqa: F401  (bass types via bacc)
import concourse.mybir as mybir
import concourse.tile as tile
from concourse import bacc
from concourse.bass_utils import run_bass_kernel_spmd

F32 = mybir.dt.float32
F16 = mybir.dt.float16

B = 8          # batch
N = 512        # segments
D = 1024       # D_in == D_out
L = 16         # linears
NCORES = 8
KC = 8         # contraction chunks of 128
MSLICE = 128   # out-dim rows per core
WMAX = 512     # PSUM window width (one bank of fp32)


def _windows(F_l, bounds):
    """Split [0, F_l) at region bounds and into balanced <=WMAX chunks.
    Returns list of (region_index, start, len) with start relative to
    the region start."""
    out = []
    for ri in range(len(bounds) - 1):
        lo, hi = bounds[ri], min(bounds[ri + 1], F_l)
        if hi <= lo:
            break
        width = hi - lo
        nw = -(-width // WMAX)
        base, rem = divmod(width, nw)
        off = 0
        for j in range(nw):
            ln = base + (1 if j < rem else 0)
            out.append((ri, lo - bounds[ri] + off, ln))
            off += ln
    return out


def _build_program(F_ls, bounds):
    """F_ls: per-linear flat-column prefix lengths (ascending process
    order, zeros removed).  bounds: region boundaries [0, b1, .., F]."""
    nc = bacc.Bacc("TRN2", target_bir_lowering=False, debug=False)
    nreg = len(bounds) - 1
    widths = [bounds[i + 1] - bounds[i] for i in range(nreg)]
    F = bounds[-1]
    NL = len(F_ls)
    total_cols = sum(F_ls)

    xs = [
        nc.dram_tensor(f"x{r}", (KC, 128, widths[r]), F16, kind="ExternalInput")
        for r in range(nreg)
    ]
    wh = nc.dram_tensor("w", (NL, 128, KC * 128), F16, kind="ExternalInput")
    y = nc.dram_tensor("y", (128, total_cols), F16, kind="ExternalOutput")

    with tile.TileContext(nc) as tc, ExitStack() as ctx:
        xpool = ctx.enter_context(tc.tile_pool(name="x", bufs=1))
        wpool = ctx.enter_context(tc.tile_pool(name="w", bufs=1))
        ypool = ctx.enter_context(tc.tile_pool(name="y", bufs=3))
        ppool = ctx.enter_context(tc.tile_pool(name="ps", bufs=8, space="PSUM"))

        # PE clock warm-up: the p-state ramps only under sustained matmul
        # activity, so burn a few dummy matmuls on scratch tiles while the
        # first real inputs are still in flight.
        sx = xpool.tile([128, 256], F16, tag="scratch_x", name="sx")
        sw = xpool.tile([128, 128], F16, tag="scratch_w", name="sw")
        nc.vector.memset(sx[:], 0.0)
        nc.vector.memset(sw[:], 0.0)
        spt = ppool.tile([128, WMAX], F32, tag="ps", name="spt")
        for _ in range(4):
            nc.tensor.matmul(spt[:, :256], sw[:], sx[:], start=True, stop=True)

        # X: one fat DMA per region on the sync HWDGE ring (per-DMA fixed
        # cost ~0.6us; transfers serialize per ring, and the two HWDGE
        # rings together saturate the 16 SDMA engines -> order transfers
        # globally by when compute needs them).  Region 0's k=0 slice
        # goes first so compute starts early.  W splits: the first few
        # linears' weights race ahead on the scalar ring; the rest queue
        # BEHIND the X regions on the sync ring so X keeps bandwidth
        # priority through the critical first ~30us.
        NW_EARLY = min(7, NL)
        xt = {}
        for r in range(nreg):
            xt[r] = xpool.tile([128, KC, widths[r]], F16, tag=f"x{r}", name=f"x{r}")
        xa0 = xpool.tile([128, widths[0]], F16, tag="xa0", name="xa0")
        wt = {}
        for li in range(NL):
            wt[li] = wpool.tile([128, KC * 128], F16, tag=f"w{li}", name=f"w{li}")
        src0 = xs[0].ap().rearrange("kc kp w -> kp kc w")
        nc.sync.dma_start(xa0[:], src0[:, 0])
        nc.sync.dma_start(xt[0][:, 1:], src0[:, 1:])
        for r in range(1, nreg):
            nc.sync.dma_start(
                xt[r][:], xs[r].ap().rearrange("kc kp w -> kp kc w")
            )
        for li in range(NL):
            if li < NW_EARLY:
                nc.scalar.dma_start(wt[li][:], wh.ap()[li])
            else:
                nc.sync.dma_start(wt[li][:], wh.ap()[li])

        def moving(ri, k, ws, wl):
            if ri == 0 and k == 0:
                return xa0[:, ws : ws + wl]
            return xt[ri][:, k, ws : ws + wl]

        off = 0
        for li in range(NL):
            F_l = F_ls[li]
            wins = _windows(F_l, bounds)
            yt = ypool.tile([128, F_l], F16, tag="yt", name=f"y{li}")
            # groups of <=4 windows: k-outer within a group (stationary
            # reuse, PSUM half-rotation so copies overlap next group)
            ydrain = 0
            for g0 in range(0, len(wins), 4):
                grp = wins[g0 : g0 + 4]
                pts = [
                    ppool.tile([128, WMAX], F32, tag="ps", name=f"ps{li}_{g0}_{j}")
                    for j in range(len(grp))
                ]
                for k in range(KC):
                    for (ri, ws, wl), pt in zip(grp, pts):
                        nc.tensor.matmul(
                            pt[:, :wl],
                            wt[li][:, k * 128 : (k + 1) * 128],
                            moving(ri, k, ws, wl),
                            start=(k == 0),
                            stop=(k == KC - 1),
                        )
                for (ri, ws, wl), pt in zip(grp, pts):
                    woff = bounds[ri] + ws
                    nc.vector.tensor_copy(yt[:, woff : woff + wl], pt[:, :wl])
                # drain completed chunks early so the tail DMA is small
                done = bounds[grp[-1][0]] + grp[-1][1] + grp[-1][2]
                if F_l >= 2000 and done < F_l and done - ydrain >= 1024:
                    nc.gpsimd.dma_start(
                        y.ap()[:, off + ydrain : off + done],
                        yt[:, ydrain:done],
                    )
                    ydrain = done
            # the last block's tail drain goes on the (idle by then)
            # HWDGE sync ring: lower fixed latency off the critical tail
            eng = nc.sync if li == NL - 1 else nc.gpsimd
            eng.dma_start(y.ap()[:, off + ydrain : off + F_l], yt[:, ydrain:])
            off += F_l

    nc.compile()
    return nc


# ---------------------------------------------------------------------------
# host wrapper
# ---------------------------------------------------------------------------

def _segment_structure(idx, T):
    t = np.arange(T)
    seg = np.searchsorted(idx, t, side="left")
    valid = seg < N
    segc = np.clip(seg, 0, N - 1)
    start = np.where(segc > 0, idx[np.maximum(segc - 1, 0)] + 1, 0)
    lidx = np.minimum(t - start, L - 1).astype(np.int64)
    lens = np.bincount(segc[valid], minlength=N)
    return t, seg, valid, segc, lidx, lens


def _install_ntff_hook():
    """Profiling-only: register the axon NTFF profile hook (dev use)."""
    import sys
    import types

    try:
        import antenv

        if "antenv.axon_hooks" not in sys.modules:
            mod = types.ModuleType("antenv.axon_hooks")
            holder = [None]
            mod.set_axon_ntff_profile_hook = lambda h: holder.__setitem__(0, h)
            mod.get_axon_ntff_profile_hook = lambda: holder[0]
            sys.modules["antenv.axon_hooks"] = mod
            antenv.axon_hooks = mod
            from trn_agent_boot.trn_boot import _ntff_profile_via_ctypes

            mod.set_axon_ntff_profile_hook(
                _ntff_profile_via_ctypes("/opt/axon/libaxon_pjrt.so")
            )
    except Exception as e:
        print(f"NTFF hook install failed: {e}")


def kernel(pooled_vectors, W, pooling_indices, target_length, _trace=False):
    pooled = np.asarray(pooled_vectors, dtype=np.float32)
    Wf = np.asarray(W, dtype=np.float32)
    idx = np.asarray(pooling_indices).astype(np.int64)
    T = int(np.asarray(target_length))

    t, seg, valid, segc, lidx, lens = _segment_structure(idx, T)

    order = np.argsort(-lens, kind="stable")      # segments by len desc
    rank_of_seg = np.empty(N, dtype=np.int64)
    rank_of_seg[order] = np.arange(N)
    N_l = (lens[None, :] > np.arange(L)[:, None]).sum(axis=1)  # (L,)

    # process order: ascending prefix length (early compute only needs
    # early X chunks), except the smallest linear moves to the end so
    # the final output drain is tiny; zero-size linears skipped
    proc = [l for l in np.argsort(N_l, kind="stable") if N_l[l] > 0]
    if len(proc) > 2:
        proc = proc[1:] + proc[:1]
    F_ls = [8 * int(N_l[l]) for l in proc]
    F = max(F_ls) if F_ls else 8
    # region boundaries: first ~4 processed linears entirely in region
    # A, ~12 in A+B
    srt = sorted(F_ls)
    cand = {F}
    if len(srt) > 4:
        cand.add(srt[3])
    if len(srt) > 12:
        cand.add(srt[11])
    bounds = [0] + sorted(cand)

    nc = _build_program(F_ls, bounds)

    # flat column axis: (rank-major, batch-minor), ranks with len>0 only
    n0 = F // 8
    Xh = (
        pooled.transpose(2, 1, 0)[:, order[:n0], :]
        .reshape(D, F)
        .astype(np.float16)
    )  # (D, F), col = r*8 + b
    xregions = [
        np.ascontiguousarray(
            Xh[:, bounds[r] : bounds[r + 1]].reshape(KC, 128, -1)
        )
        for r in range(len(bounds) - 1)
    ]

    in_maps = []
    for c in range(NCORES):
        # W[l, m_slice, d] -> (l, kp, kc*128+m) with d = kc*128 + kp
        wc = (
            Wf[np.array(proc), c * 128 : (c + 1) * 128, :]
            .transpose(0, 2, 1)
            .reshape(len(proc), KC, 128, 128)
            .transpose(0, 2, 1, 3)
            .reshape(len(proc), 128, KC * 128)
            .astype(np.float16)
        )
        im = {"w": np.ascontiguousarray(wc)}
        for r, xr in enumerate(xregions):
            im[f"x{r}"] = xr
        in_maps.append(im)

    kwargs = {}
    if _trace:
        _install_ntff_hook()
        kwargs = dict(trace=True)
    res = run_bass_kernel_spmd(nc, in_maps, core_ids=list(range(NCORES)), **kwargs)
    results = res.results

    # assemble (D, total_cols) then scatter to (B, T, D)
    Yall = np.concatenate(
        [np.asarray(results[c]["y"]) for c in range(NCORES)], axis=0
    )  # (1024, total_cols) f16
    col_off = np.zeros(L, dtype=np.int64)
    off = 0
    for li, l in enumerate(proc):
        col_off[l] = off
        off += F_ls[li]

    Dout = Wf.shape[1]
    out = np.zeros((B, T, Dout), dtype=np.float32)
    tv = t[valid]
    ci = col_off[lidx[tv]] + rank_of_seg[segc[tv]] * 8  # (Tv,)
    cib = ci[:, None] + np.arange(B)[None, :]           # (Tv, B)
    out[:, tv, :] = Yall[:, cib].transpose(2, 1, 0).astype(np.float32)

    if _trace:
        kernel._last_exec_time_ns = res.exec_time_ns
        kernel._last_results = res
    return out
